# revision 31
# baseline (speedup 1.0000x reference)
"""RNN-T Joiner kernel for Trainium2, data-parallel over (B, T) on 8 cores.

reference:
    logit = tanh(enc[:, :, None, :] + dec[:, None, :, :])   # (B,T,U,C)
    out   = einsum('btuc,vc->btuv', logit, W) + b           # (B,T,U,V)

Shapes (hardcoded): B=4, T=256, U=64, C=512, V=1024.

Sharding: core k handles b = k//2, t rows [ (k%2)*128, (k%2)*128+128 ).
W / bias replicated. No collectives.

Per-core device kernel (C on partitions for the logit):
  - all inputs host-prepacked to contiguous [128, X] SBUF layouts; input
    DMAs split across sync/scalar/gpsimd queues so descriptor generation
    overlaps.
  - warmup matmuls on a memset junk tile run during the input-DMA
    preamble so the PE HAM clock-gate is released (2.4 GHz) before the
    real matmul stream starts.
  - logitT[c, t] = tanh(encT[c, t] + decT[c, u]) on the scalar engine
    (fused per-partition bias add), output cast to bf16.
  - out[t, v] accumulated over 4 c-chunks of K=128 bf16 matmuls
    (bf16 enables fast weight load; PSUM accumulation stays fp32).
  - bias add fused into the PSUM->SBUF eviction on DVE.
  - out tile DMA'd straight to DRAM (4KB contiguous per partition).
"""

import numpy as np


def _ensure_ntff_hook():
    """Make BASS_TRACE work when the image's `antenv` lacks `axon_hooks`.

    bass_utils' axon trace path imports antenv.axon_hooks; some images
    ship antenv without that submodule, so tracing silently degrades.
    Install a minimal module providing the get/set pair, wired to the
    ctypes NTFF hook from trn_agent_boot when available. Purely
    additive: no-op if the real module exists.
    """
    try:
        from antenv.axon_hooks import get_axon_ntff_profile_hook  # noqa: F401

        return
    except ImportError:
        pass
    import sys
    import types

    hook = None
    try:
        from trn_agent_boot.trn_boot import _ntff_profile_via_ctypes

        hook = _ntff_profile_via_ctypes("/opt/axon/libaxon_pjrt.so")
    except Exception:
        hook = None
    mod = types.ModuleType("antenv.axon_hooks")
    mod._hook = hook
    mod.get_axon_ntff_profile_hook = lambda: mod._hook

    def _set(h):
        mod._hook = h

    mod.set_axon_ntff_profile_hook = _set
    sys.modules["antenv.axon_hooks"] = mod
    try:
        import antenv

        antenv.axon_hooks = mod
    except ImportError:
        pass


B, T, U, C, V = 4, 256, 64, 512, 1024
NCORES = 8
TS = 128  # t rows per core
CCH = C // 128  # 4 contraction chunks
VH = V // 512  # 2 psum-width chunks
NWARM = 9  # HAM warmup matmuls

_CACHE = {}


def _build():
    from contextlib import ExitStack

    import concourse.bacc as bacc
    import concourse.mybir as mybir
    import concourse.tile as tile

    dt = mybir.dt
    f32 = dt.float32
    bf16 = dt.bfloat16

    nc = bacc.Bacc("TRN2", target_bir_lowering=False, debug=False, num_devices=NCORES)
    # inputs prepacked host-side to the exact SBUF layouts; the pieces
    # needed first (enc, dec, wt c0) ride in one blob so the preamble
    # DMA uses fat 3.5KB-per-partition descriptors (descriptor-rate is
    # the preamble bottleneck, not bytes)
    EB, DB = CCH * TS, CCH * U  # 512, 256 cols
    AW = EB + DB + V  # blob_a cols: enc | dec | wt c0
    blob_a = nc.declare_dram_parameter("blob_a", [128, AW], bf16, isOutput=False)
    wt_rest = nc.declare_dram_parameter("wt_rest", [128, 3 * V], bf16, isOutput=False)
    bias_rep = nc.declare_dram_parameter("bias_rep", [128, V], bf16, isOutput=False)
    out = nc.declare_dram_parameter("out", [TS, U, V], f32, isOutput=True)

    with tile.TileContext(nc) as tc, ExitStack() as ctx:
        const = ctx.enter_context(tc.tile_pool(name="const", bufs=1))
        logit_pool = ctx.enter_context(tc.tile_pool(name="logit", bufs=6))
        psum_pool = ctx.enter_context(tc.tile_pool(name="psum", bufs=4, space="PSUM"))
        out_pool = ctx.enter_context(tc.tile_pool(name="out", bufs=8))

        ab = const.tile([128, AW], bf16, tag="ab")
        wtr = const.tile([128, 3 * V], bf16, tag="wtr")
        bias_bf = const.tile([128, V], bf16, tag="bias_bf")
        bias_sb = const.tile([128, V], f32, tag="bias")
        wj = const.tile([128, 512], bf16, tag="wj")
        scr = const.tile([128, 1], bf16, tag="scr")

        # junk operand for PE warmup; DVE's first real work (the bias
        # CAST) is late, so an early memset here is free
        nc.vector.memset(wj[:], 0.0)
        # dependency-free dummy activation: forces ACT_TABLE_LOAD to run
        # during the preamble instead of before the first real tanh
        nc.scalar.activation(
            scr[:],
            wj[:, 0:1],
            mybir.ActivationFunctionType.Tanh,
            bias=wj[:, 1:2],
        )
        for i in range(NWARM):
            wps = psum_pool.tile([128, V], f32, tag="ps")
            nc.tensor.matmul(
                wps[:, :512], lhsT=wj[:, :128], rhs=wj[:], start=True, stop=True
            )

        # all input DMAs on the sync ring, in consumption order; the
        # critical blob goes first and later chunks arrive just in time
        nc.sync.dma_start(ab[:], blob_a[:])
        for c in range(3):
            nc.sync.dma_start(
                wtr[:, c * V : (c + 1) * V], wt_rest[:, c * V : (c + 1) * V]
            )
        nc.sync.dma_start(bias_bf[:], bias_rep[:])
        # one-time widen; DVE is idle during the preamble
        nc.vector.tensor_copy(bias_sb[:], bias_bf[:])

        def rhs_view(c, vh):
            if c == 0:
                return ab[:, EB + DB + vh * 512 : EB + DB + vh * 512 + 512]
            return wtr[:, (c - 1) * V + vh * 512 : (c - 1) * V + vh * 512 + 512]

        def make_logit(u):
            lg = logit_pool.tile([128, CCH * TS], bf16, tag="lg")
            for c in range(CCH):
                nc.scalar.activation(
                    lg[:, c * TS : (c + 1) * TS],
                    ab[:, c * TS : (c + 1) * TS],
                    mybir.ActivationFunctionType.Tanh,
                    bias=ab[:, EB + c * U + u : EB + c * U + u + 1],
                )
            return lg

        def matmuls(ps, off, lg):
            for c in range(CCH):
                for vh in range(VH):
                    nc.tensor.matmul(
                        ps[:, off + vh * 512 : off + (vh + 1) * 512],
                        lhsT=lg[:, c * TS : (c + 1) * TS],
                        rhs=rhs_view(c, vh),
                        start=(c == 0),
                        stop=(c == CCH - 1),
                    )

        for u in range(U):
            ps = psum_pool.tile([128, V], f32, tag="ps")
            matmuls(ps, 0, make_logit(u))
            ob = out_pool.tile([128, V], f32, tag="ob")
            nc.vector.tensor_add(ob[:], ps[:], bias_sb[:])
            nc.sync.dma_start(out[:, u, :], ob[:])

    nc.finalize()
    return nc


def _get_nc():
    if "nc" not in _CACHE:
        _CACHE["nc"] = _build()
    return _CACHE["nc"]


def _chunk128(a):
    """[D, X] -> [128, (D//128)*X] with chunk-major free dim."""
    d, x = a.shape
    return np.ascontiguousarray(
        a.reshape(d // 128, 128, x).transpose(1, 0, 2).reshape(128, (d // 128) * x)
    )


def kernel(**inputs):
    import ml_dtypes

    enc = np.asarray(inputs["enc_out"], dtype=np.float32)
    dec = np.asarray(inputs["dec_out"], dtype=np.float32)
    W = np.asarray(inputs["W"], dtype=np.float32)
    b = np.asarray(inputs["b"], dtype=np.float32)

    nc = _get_nc()

    bf = ml_dtypes.bfloat16
    wt_np = _chunk128(np.ascontiguousarray(W.T)).astype(bf)
    bias_np = np.ascontiguousarray(np.broadcast_to(b, (128, V))).astype(bf)
    in_maps = []
    for k in range(NCORES):
        bb, t0 = k // 2, (k % 2) * TS
        enc_p = _chunk128(np.ascontiguousarray(enc[bb, t0 : t0 + TS, :].T)).astype(bf)
        dec_p = _chunk128(np.ascontiguousarray(dec[bb].T)).astype(bf)
        in_maps.append(
            {
                "blob_a": np.ascontiguousarray(
                    np.concatenate([enc_p, dec_p, wt_np[:, :V]], axis=1)
                ),
                "wt_rest": np.ascontiguousarray(wt_np[:, V:]),
                "bias_rep": bias_np,
            }
        )

    _ensure_ntff_hook()
    from concourse.bass_utils import run_bass_kernel_spmd

    res = run_bass_kernel_spmd(nc, in_maps, list(range(NCORES)))
    _CACHE["last_result"] = res

    out = np.empty((B, T, U, V), np.float32)
    for k in range(NCORES):
        bb, t0 = k // 2, (k % 2) * TS
        out[bb, t0 : t0 + TS] = res.results[k]["out"]
    return out


# revision 34
# speedup vs baseline: 1.0238x; 1.0238x over previous
"""RNN-T Joiner kernel for Trainium2, data-parallel over (B, T) on 8 cores.

reference:
    logit = tanh(enc[:, :, None, :] + dec[:, None, :, :])   # (B,T,U,C)
    out   = einsum('btuc,vc->btuv', logit, W) + b           # (B,T,U,V)

Shapes (hardcoded): B=4, T=256, U=64, C=512, V=1024.

Sharding: core k handles b = k//2, t rows [ (k%2)*128, (k%2)*128+128 ).
W / bias replicated. No collectives.

Per-core device kernel (C on partitions for the logit):
  - all inputs host-prepacked bf16 in contiguous [128, X] SBUF layouts;
    the critical pieces (enc | dec | wt c0) ride one blob DMA with fat
    3.5KB-per-partition descriptors (the preamble is descriptor-rate
    bound); remaining wt chunks + bias follow in consumption order, all
    on the sync ring (sustained descriptor-gen on gpsimd throttles the
    PE clock).
  - 9 warmup matmuls on a memset junk tile during the input-DMA
    preamble release the PE HAM clock-gate (1.2 -> 2.4 GHz) and seam
    directly into the real stream; a dependency-free dummy activation
    forces ACT_TABLE_LOAD off the critical path.
  - logitT[c, t] = tanh(encT[c, t] + decT[c, u]) on the scalar engine
    (fused per-partition bias add), output cast to bf16.
  - out[t, v] accumulated over 4 c-chunks of K=128 bf16 matmuls (bf16
    streams 1 col/cycle like fp32 but gets fast weight load -> warm
    matmuls at the 216 ns issue floor; PSUM accumulation stays fp32).
  - bias add fused into the PSUM->SBUF eviction on DVE.
  - out tile DMA'd straight to DRAM (4KB contiguous per partition).

Measured on trn2 (8 cores): ~129-131 us vs 137-139 us for the fp32
baseline; warm matmul stream runs gap-free at 216 ns/MM (N=512 issue
floor), rel err ~2.2e-3 (tolerance 2e-2).
"""

import numpy as np


def _ensure_ntff_hook():
    """Make BASS_TRACE work when the image's `antenv` lacks `axon_hooks`.

    bass_utils' axon trace path imports antenv.axon_hooks; some images
    ship antenv without that submodule, so tracing silently degrades.
    Install a minimal module providing the get/set pair, wired to the
    ctypes NTFF hook from trn_agent_boot when available. Purely
    additive: no-op if the real module exists.
    """
    try:
        from antenv.axon_hooks import get_axon_ntff_profile_hook  # noqa: F401

        return
    except ImportError:
        pass
    import sys
    import types

    hook = None
    try:
        from trn_agent_boot.trn_boot import _ntff_profile_via_ctypes

        hook = _ntff_profile_via_ctypes("/opt/axon/libaxon_pjrt.so")
    except Exception:
        hook = None
    mod = types.ModuleType("antenv.axon_hooks")
    mod._hook = hook
    mod.get_axon_ntff_profile_hook = lambda: mod._hook

    def _set(h):
        mod._hook = h

    mod.set_axon_ntff_profile_hook = _set
    sys.modules["antenv.axon_hooks"] = mod
    try:
        import antenv

        antenv.axon_hooks = mod
    except ImportError:
        pass


B, T, U, C, V = 4, 256, 64, 512, 1024
NCORES = 8
TS = 128  # t rows per core
CCH = C // 128  # 4 contraction chunks
VH = V // 512  # 2 psum-width chunks
NWARM = 9  # HAM warmup matmuls

_CACHE = {}


def _build():
    from contextlib import ExitStack

    import concourse.bacc as bacc
    import concourse.mybir as mybir
    import concourse.tile as tile

    dt = mybir.dt
    f32 = dt.float32
    bf16 = dt.bfloat16

    nc = bacc.Bacc("TRN2", target_bir_lowering=False, debug=False, num_devices=NCORES)
    # inputs prepacked host-side to the exact SBUF layouts; the pieces
    # needed first (enc, dec, wt c0) ride in one blob so the preamble
    # DMA uses fat 3.5KB-per-partition descriptors (descriptor-rate is
    # the preamble bottleneck, not bytes)
    EB, DB = CCH * TS, CCH * U  # 512, 256 cols
    AW = EB + DB + V  # blob_a cols: enc | dec | wt c0
    blob_a = nc.declare_dram_parameter("blob_a", [128, AW], bf16, isOutput=False)
    wt_rest = nc.declare_dram_parameter("wt_rest", [128, 3 * V], bf16, isOutput=False)
    bias_rep = nc.declare_dram_parameter("bias_rep", [128, V], bf16, isOutput=False)
    out = nc.declare_dram_parameter("out", [TS, U, V], f32, isOutput=True)

    with tile.TileContext(nc) as tc, ExitStack() as ctx:
        const = ctx.enter_context(tc.tile_pool(name="const", bufs=1))
        logit_pool = ctx.enter_context(tc.tile_pool(name="logit", bufs=6))
        psum_pool = ctx.enter_context(tc.tile_pool(name="psum", bufs=4, space="PSUM"))
        out_pool = ctx.enter_context(tc.tile_pool(name="out", bufs=8))

        ab = const.tile([128, AW], bf16, tag="ab")
        wtr = const.tile([128, 3 * V], bf16, tag="wtr")
        bias_bf = const.tile([128, V], bf16, tag="bias_bf")
        bias_sb = const.tile([128, V], f32, tag="bias")
        wj = const.tile([128, 512], bf16, tag="wj")
        scr = const.tile([128, 1], bf16, tag="scr")

        # junk operand for PE warmup; DVE's first real work (the bias
        # CAST) is late, so an early memset here is free
        nc.vector.memset(wj[:], 0.0)
        # dependency-free dummy activation: forces ACT_TABLE_LOAD to run
        # during the preamble instead of before the first real tanh
        nc.scalar.activation(
            scr[:],
            wj[:, 0:1],
            mybir.ActivationFunctionType.Tanh,
            bias=wj[:, 1:2],
        )
        for i in range(NWARM):
            wps = psum_pool.tile([128, V], f32, tag="ps")
            nc.tensor.matmul(
                wps[:, :512], lhsT=wj[:, :128], rhs=wj[:], start=True, stop=True
            )

        # all input DMAs on the sync ring, in consumption order; the
        # critical blob goes first and later chunks arrive just in time
        nc.sync.dma_start(ab[:], blob_a[:])
        for c in range(3):
            nc.sync.dma_start(
                wtr[:, c * V : (c + 1) * V], wt_rest[:, c * V : (c + 1) * V]
            )
        nc.sync.dma_start(bias_bf[:], bias_rep[:])
        # one-time widen; DVE is idle during the preamble
        nc.vector.tensor_copy(bias_sb[:], bias_bf[:])

        def rhs_view(c, vh):
            if c == 0:
                return ab[:, EB + DB + vh * 512 : EB + DB + vh * 512 + 512]
            return wtr[:, (c - 1) * V + vh * 512 : (c - 1) * V + vh * 512 + 512]

        def make_logit(u):
            lg = logit_pool.tile([128, CCH * TS], bf16, tag="lg")
            for c in range(CCH):
                nc.scalar.activation(
                    lg[:, c * TS : (c + 1) * TS],
                    ab[:, c * TS : (c + 1) * TS],
                    mybir.ActivationFunctionType.Tanh,
                    bias=ab[:, EB + c * U + u : EB + c * U + u + 1],
                )
            return lg

        def matmuls(ps, off, lg):
            for c in range(CCH):
                for vh in range(VH):
                    nc.tensor.matmul(
                        ps[:, off + vh * 512 : off + (vh + 1) * 512],
                        lhsT=lg[:, c * TS : (c + 1) * TS],
                        rhs=rhs_view(c, vh),
                        start=(c == 0),
                        stop=(c == CCH - 1),
                    )

        for u in range(U):
            ps = psum_pool.tile([128, V], f32, tag="ps")
            matmuls(ps, 0, make_logit(u))
            ob = out_pool.tile([128, V], f32, tag="ob")
            nc.vector.tensor_add(ob[:], ps[:], bias_sb[:])
            nc.sync.dma_start(out[:, u, :], ob[:])

    nc.finalize()
    return nc


def _get_nc():
    if "nc" not in _CACHE:
        _CACHE["nc"] = _build()
    return _CACHE["nc"]


def _chunk128(a):
    """[D, X] -> [128, (D//128)*X] with chunk-major free dim."""
    d, x = a.shape
    return np.ascontiguousarray(
        a.reshape(d // 128, 128, x).transpose(1, 0, 2).reshape(128, (d // 128) * x)
    )


def kernel(**inputs):
    import ml_dtypes

    enc = np.asarray(inputs["enc_out"], dtype=np.float32)
    dec = np.asarray(inputs["dec_out"], dtype=np.float32)
    W = np.asarray(inputs["W"], dtype=np.float32)
    b = np.asarray(inputs["b"], dtype=np.float32)

    nc = _get_nc()

    bf = ml_dtypes.bfloat16
    wt_np = _chunk128(np.ascontiguousarray(W.T)).astype(bf)
    bias_np = np.ascontiguousarray(np.broadcast_to(b, (128, V))).astype(bf)
    in_maps = []
    for k in range(NCORES):
        bb, t0 = k // 2, (k % 2) * TS
        enc_p = _chunk128(np.ascontiguousarray(enc[bb, t0 : t0 + TS, :].T)).astype(bf)
        dec_p = _chunk128(np.ascontiguousarray(dec[bb].T)).astype(bf)
        in_maps.append(
            {
                "blob_a": np.ascontiguousarray(
                    np.concatenate([enc_p, dec_p, wt_np[:, :V]], axis=1)
                ),
                "wt_rest": np.ascontiguousarray(wt_np[:, V:]),
                "bias_rep": bias_np,
            }
        )

    _ensure_ntff_hook()
    from concourse.bass_utils import run_bass_kernel_spmd

    res = run_bass_kernel_spmd(nc, in_maps, list(range(NCORES)))
    _CACHE["last_result"] = res

    out = np.empty((B, T, U, V), np.float32)
    for k in range(NCORES):
        bb, t0 = k // 2, (k % 2) * TS
        out[bb, t0 : t0 + TS] = res.results[k]["out"]
    return out
